# revision 32
# baseline (speedup 1.0000x reference)
"""Multi-head attention (B=2, N=2048, D=1024, H=16) on 8 TRN2 NeuronCores.

Sharding: tensor-parallel over heads across all 8 cores. Core i handles
heads [2i, 2i+2) (128 of the 1024 QKV output dims) for BOTH batches.
After local attention, an 8-core AllToAll (split in two, one per local
head, so the first overlaps the second head's attention) re-shards from
head-split to (batch, sequence-slab)-split; core j then computes the
output projection for batch j//4, rows [512*(j%4), 512*(j%4+1)).
Host-side work is slicing/layout only (x is passed transposed [D, N] per
batch in fp16 — the shard layout the device consumes directly).

Device layout notes:
  - fp16 everywhere (vs bf16) buys error headroom; Q,K are additionally
    quantized to fp8e4 so the QK^T matmuls run in DoubleRow perf mode at
    2x PE rate. DoubleRow contracts two 64-partition subtiles per
    instruction; subtile 1 of QZ/KZ is a zero plane, so the 64-dim head
    contraction maps onto it without any partition-fold DMA.
  - The 1/sqrt(hd) score scale is folded into the exp activation's scale
    operand, so Q/K biases are applied raw (better fp8 utilization).
  - Q/K/V bias adds and all downcasts run on DVE, keeping the Activation
    engine free for the exp stream, which is the phase floor (~1ns/col).
  - Scores are computed transposed (S^T [k, q]) so exp(S^T) tiles feed the
    AV matmul as the moving operand with k on partitions.
  - V gets an appended ones column per head, so the AV matmul also emits
    softmax denominators (row 64 of the [65, q] accumulator) for free.
  - QKV work for batch 1 (and V/Q slabs of batch 0) is emitted interleaved
    into attention's per-chunk schedule slots, filling the PE bubbles that
    the exp-paced pipeline would otherwise leave.
"""

import ml_dtypes
import numpy as np

import concourse.bass as bass
import concourse.mybir as mybir
import concourse.tile as tile
from concourse import bacc
from concourse.bass_utils import run_bass_kernel_spmd
from concourse.masks import make_identity

F32 = mybir.dt.float32
F16 = mybir.dt.float16
F8 = mybir.dt.float8e4
EXP = mybir.ActivationFunctionType.Exp
BYPASS = mybir.AluOpType.bypass
DR = mybir.MatmulPerfMode.DoubleRow

P = 128
B, N, D = 2, 2048, 1024
NH, HD = 16, 64
TP = 8                  # head-parallel group size (all cores)
HL = D // TP            # 128 local head dims (2 heads x 64)
NHL = NH // TP          # 2 local heads
QS = 512                # query slab width
NQS = N // QS           # 4 slabs
KC = N // P             # 16 key chunks of 128
DC = D // P             # 8 model-dim chunks of 128
ROWS = 512              # output rows per core (one slab of one batch)
RG = [[0, 1, 2, 3, 4, 5, 6, 7]]
SCALE = 1.0 / np.sqrt(HD)

_CACHE = {}


def build_nc(reps=1):
    nc = bacc.Bacc("TRN2", target_bir_lowering=False, debug=False,
                   num_devices=8)

    xt_ext = nc.declare_dram_parameter("xt", [B, D, N], F16, isOutput=False)
    xt8_ext = nc.declare_dram_parameter("xt8", [B, D, N], F8, isOutput=False)
    wq_ext = nc.declare_dram_parameter("wq", [P, DC, HL], F16, isOutput=False)
    bq_ext = nc.declare_dram_parameter("bq", [P, 1], F32, isOutput=False)
    wk_ext = nc.declare_dram_parameter("wk", [P, DC, HL], F16, isOutput=False)
    bk_ext = nc.declare_dram_parameter("bk", [P, 1], F32, isOutput=False)
    wv_ext = nc.declare_dram_parameter("wv", [P, DC, HL], F16, isOutput=False)
    bv_ext = nc.declare_dram_parameter("bv", [1, HL], F32, isOutput=False)
    wo_ext = nc.declare_dram_parameter("wo", [P, DC, D], F16, isOutput=False)
    bo_ext = nc.declare_dram_parameter("bo", [1, D], F32, isOutput=False)
    out_ext = nc.declare_dram_parameter("out", [ROWS, D], F16, isOutput=True)

    with tile.TileContext(nc) as tc:
        with (
            tc.tile_pool(name="const", bufs=1) as const,
            tc.tile_pool(name="persist", bufs=1) as persist,
            tc.tile_pool(name="dram", bufs=1, space="DRAM") as dram,
            tc.tile_pool(name="xtp", bufs=2) as xtp,
            tc.tile_pool(name="xtp8", bufs=2) as xtp8,
            tc.tile_pool(name="wp", bufs=1) as wp,
            tc.tile_pool(name="vtp", bufs=3) as vtp,
            tc.tile_pool(name="wo_p", bufs=1) as wo_p,
            tc.tile_pool(name="ptp", bufs=6) as ptp,
            tc.tile_pool(name="nrm", bufs=4) as nrm,
            tc.tile_pool(name="psA", bufs=2, space="PSUM") as psA,
            tc.tile_pool(name="psB", bufs=2, space="PSUM") as psB,
            tc.tile_pool(name="psV", bufs=2, space="PSUM") as psV,
        ):
            identity_h = const.tile([P, P], F16)
            make_identity(nc, identity_h)

            # persistent SBUF tensors
            # QZ/KZ: fp8 Q^T/K^T with a zero subtile plane for DoubleRow
            QZ = persist.tile([P, B, 2, N], F8)       # [p, b, sub, q]
            KZ = persist.tile([P, B, 2, N], F8)
            nc.gpsimd.memset(QZ[:, :, 1, :], 0.0)
            nc.gpsimd.memset(KZ[:, :, 1, :], 0.0)
            Vaug = persist.tile([P, B, KC, NHL, HD + 1], F16)
            nc.gpsimd.memset(Vaug[:, :, :, :, HD:HD + 1], 1.0)

            # h0 travels in one AllToAll; h1 in two column-halved AllToAlls
            # so the first can fire mid-attention and the O-proj rows it
            # feeds overlap the second collective's flight.
            a2a_in0 = dram.tile([TP, HD, QS], F16, name="a2a_in0")
            a2a_out0 = dram.tile([TP, HD, QS], F16, name="a2a_out0")
            a2a_in1 = [dram.tile([TP, HD, QS // 2], F16, name=f"a2a_in1{c}")
                       for c in range(2)]
            a2a_out1 = [dram.tile([TP, HD, QS // 2], F16, name=f"a2a_out1{c}")
                        for c in range(2)]

            xts = {}
            xt8s = {}

            def emit_xt(b):
                # f16 x for the V path, [128, 1024] half-slab chunks
                xts[b] = xtp.tile([P, DC, N], F16, tag="xT", name=f"xT{b}")
                xT = xts[b]
                for qh in range(2):
                    for dc in range(DC):
                        nc.sync.dma_start(
                            xT[:, dc, qh * (N // 2):(qh + 1) * (N // 2)],
                            xt_ext[b, dc * P:(dc + 1) * P,
                                   qh * (N // 2):(qh + 1) * (N // 2)])

            def emit_xt8(b):
                # fp8 x feeds only the Q/K projections: half the bytes on
                # the startup-critical DMA path
                xt8s[b] = xtp8.tile([P, DC, N], F8, tag="xT8", name=f"xT8{b}")
                xT8 = xt8s[b]
                for dp in range(DC // 2):
                    nc.sync.dma_start(
                        xT8[:, 2 * dp:2 * dp + 2, :],
                        xt8_ext[b, dp * 2 * P:(dp + 1) * 2 * P, :]
                        .rearrange("(c p) n -> p c n", p=P))

            # DMA order is queue order: the small weight/bias tensors the
            # first matmuls block on go first, then x in use order
            wq_sb = wp.tile([P, DC, HL], F16)
            wk_sb = wp.tile([P, DC, HL], F16)
            wv_sb = wp.tile([P, DC, HL], F16)
            bq_sb = wp.tile([P, 1], F32)   # raw; 1/sqrt(hd) folded into exp
            bk_sb = wp.tile([P, 1], F32)
            bv_sb = wp.tile([1, HL], F32)
            nc.sync.dma_start(wq_sb, wq_ext[:])
            nc.sync.dma_start(wk_sb, wk_ext[:])
            nc.sync.dma_start(bq_sb, bq_ext[:])
            nc.sync.dma_start(bk_sb, bk_ext[:])
            emit_xt8(0)
            nc.sync.dma_start(wv_sb, wv_ext[:])
            nc.sync.dma_start(bv_sb, bv_ext[:])
            emit_xt(0)
            emit_xt8(1)
            bv_bc = wp.tile([P, HL], F32)
            nc.gpsimd.partition_broadcast(bv_bc[:], bv_sb[:])

            def unit_kq(dstZ, w_sb, bias_sb, b, qs):
                # one projection slab: psum matmuls + DVE bias-add to fp8
                psm = psV.tile([P, QS], F32, tag="mix", name="kqpsm")
                xT8 = xt8s[b]
                for dc in range(DC):
                    nc.tensor.matmul(
                        psm, lhsT=w_sb[:, dc, :],
                        rhs=xT8[:, dc, qs * QS:(qs + 1) * QS],
                        start=(dc == 0), stop=(dc == DC - 1))
                with nc.allow_low_precision(reason="fp8 q/k for scores"):
                    nc.vector.tensor_scalar_add(
                        dstZ[:, b, 0, qs * QS:(qs + 1) * QS], psm,
                        bias_sb[:, 0:1])

            def unit_qk_pair(b, qs):
                # Q and K projections interleaved at chunk level: at startup
                # both are paced by the same xt8 DMA chunks, so interleaving
                # finishes both ~when the last chunk lands instead of 2x later
                psq = psV.tile([P, QS], F32, tag="mix", name="psq")
                psk = psB.tile([P, QS], F32, tag="acc", name="psk")
                xT8 = xt8s[b]
                for dc in range(DC):
                    for psm, w_sb in ((psq, wq_sb), (psk, wk_sb)):
                        nc.tensor.matmul(
                            psm, lhsT=w_sb[:, dc, :],
                            rhs=xT8[:, dc, qs * QS:(qs + 1) * QS],
                            start=(dc == 0), stop=(dc == DC - 1))
                with nc.allow_low_precision(reason="fp8 q/k for scores"):
                    nc.vector.tensor_scalar_add(
                        QZ[:, b, 0, qs * QS:(qs + 1) * QS], psq,
                        bq_sb[:, 0:1])
                    nc.vector.tensor_scalar_add(
                        KZ[:, b, 0, qs * QS:(qs + 1) * QS], psk,
                        bk_sb[:, 0:1])

            def unit_v(b, ks):
                # V^T slab -> PE-transpose 128x128 chunks into Vaug [k, d]
                psm = psV.tile([P, QS], F32, tag="mix", name="vpsm")
                xT = xts[b]
                for dc in range(DC):
                    nc.tensor.matmul(
                        psm, lhsT=wv_sb[:, dc, :],
                        rhs=xT[:, dc, ks * QS:(ks + 1) * QS],
                        start=(dc == 0), stop=(dc == DC - 1))
                vt_t = vtp.tile([P, QS], F16, name="vt_t")
                nc.vector.tensor_copy(vt_t, psm)
                for kk in range(QS // P):
                    kc = ks * (QS // P) + kk
                    pst = psV.tile([P, P], F16, tag="mix", name="pst")
                    nc.tensor.transpose(
                        pst, vt_t[:, kk * P:(kk + 1) * P], identity_h)
                    nc.vector.tensor_add(
                        out=Vaug[:, b, kc, :, :HD],
                        in0=pst[:].rearrange("p (h d) -> p h d", d=HD),
                        in1=bv_bc[:].rearrange("p (h d) -> p h d", d=HD))

            def attn(h, b, dst, sched=None, co=0, cw=QS, post_qs=None):
                # processes query columns [qs*QS+co, +cw) of every slab;
                # G = chunks per [128, 1024] PSUM group / exp instruction
                po = h * HD
                G = (2 * QS) // cw
                for qs in range(NQS):
                    j = b * NQS + qs      # a2a destination core
                    q0 = qs * QS + co
                    acc = psB.tile([P, cw], F32, tag="acc", name="acc")

                    def av(g, pt):
                        for i in range(G):
                            kc = g * G + i
                            nc.tensor.matmul(
                                acc[:HD + 1],
                                lhsT=Vaug[:, b, kc, h, :],
                                rhs=pt[:, i * cw:(i + 1) * cw],
                                start=(kc == 0), stop=(kc == KC - 1))

                    pend = []
                    for g in range(KC // G):
                        # G fp8 DoubleRow score chunks into one 2-bank
                        # PSUM tile so a single exp covers them all
                        pss = psA.tile([P, 2 * QS], F32, tag="pss",
                                       name="pss")
                        for i in range(G):
                            kc = g * G + i
                            nc.tensor.matmul(
                                pss[:, i * cw:(i + 1) * cw],
                                lhsT=KZ[po:po + HD, b, :,
                                        kc * P:(kc + 1) * P],
                                rhs=QZ[po:po + HD, b, :, q0:q0 + cw],
                                perf_mode=DR, start=True, stop=True)
                        pt = ptp.tile([P, 2 * QS], F16, name="pt")
                        nc.scalar.activation(pt, pss, EXP, scale=SCALE)
                        # AV lags one tile: exp(i) overlaps QK(i+1) on the PE
                        pend.append((g, pt))
                        if len(pend) >= 2:
                            av(*pend.pop(0))
                        if sched:
                            for fn in sched.pop((qs, g), ()):
                                fn()
                    while pend:
                        av(*pend.pop(0))
                    rec = nrm.tile([1, cw], F16, name="rec")
                    with nc.allow_low_precision(
                            reason="softmax denom reciprocal to f16"):
                        nc.vector.reciprocal(rec, acc[HD:HD + 1])
                    bc_sb = nrm.tile([HD, cw], F16, tag="bcsb", name="bc_sb")
                    nc.gpsimd.partition_broadcast(bc_sb[:], rec[:])
                    onrm = nrm.tile([HD, cw], F16, tag="onrm", name="onrm")
                    nc.vector.tensor_mul(onrm, acc[:HD], bc_sb)
                    nc.sync.dma_start(dst[j, :, :], onrm)
                    if post_qs and qs in post_qs:
                        post_qs[qs](onrm)

            def K_(b, qs):
                return lambda: unit_kq(KZ, wk_sb, bk_sb, b, qs)

            def Q_(b, qs):
                return lambda: unit_kq(QZ, wq_sb, bq_sb, b, qs)

            def V_(b, ks):
                return lambda: unit_v(b, ks)

            for _rep in range(reps):
                if _rep > 0:
                    emit_xt(0)
                # batch-0 Q+K slab 0 (interleaved) + K full up front; V and
                # the rest (plus batch 1's projections) fill attention's PE
                # bubbles so the first scores aren't queued behind them
                unit_qk_pair(0, 0)
                for qs in range(1, NQS):
                    unit_kq(KZ, wk_sb, bk_sb, 0, qs)
                # V_(b, ks) must be emitted before the AV matmul that
                # consumes Vaug[4ks..4ks+3] (PE queue is in-order; a
                # consumer emitted first would deadlock): AV(g) is emitted
                # after exp(g+1), so V_(.,i) sits at qs0 slot 2i-1.
                sched00 = {
                    (0, 0): [V_(0, 0)],
                    (0, 1): [V_(0, 1)], (0, 3): [V_(0, 2)],
                    (0, 5): [V_(0, 3)], (0, 7): [Q_(0, 1)],
                    (1, 1): [lambda: emit_xt(1)], (1, 3): [K_(1, 0)],
                    (1, 5): [K_(1, 1)], (1, 7): [Q_(0, 2)],
                    (2, 1): [K_(1, 2)], (2, 3): [K_(1, 3)],
                    (2, 7): [Q_(0, 3)],
                    (3, 3): [Q_(1, 0)],
                }  # K/Q(b1) read xt8 (landed by ~13us), V(b1) reads f16 x
                attn(0, 0, a2a_in0, sched00)
                unit_v(1, 0)
                # Q slabs for (h0, b1) slot just ahead of the qs that needs
                # them, moving PE load out of the overloaded attn(0,0) phase
                sched01 = {
                    (0, 1): [V_(1, 1)], (0, 3): [V_(1, 2)],
                    (0, 5): [V_(1, 3)], (0, 7): [Q_(1, 1)],
                    (1, 7): [Q_(1, 2)], (2, 7): [Q_(1, 3)],
                }
                attn(0, 1, a2a_in0, sched01)
                nc.gpsimd.collective_compute(
                    "AllToAll", BYPASS,
                    ins=[a2a_in0[:].opt()],
                    outs=[a2a_out0[:].opt()],
                    replica_groups=RG)
                # load wo late so it doesn't compete with xT DMA at start
                wo_sb = wo_p.tile([P, DC, D], F16, tag="wo_sb", name="wo_sb")
                nc.sync.dma_start(wo_sb, wo_ext[:])
                bo_sb = wo_p.tile([1, D], F32, tag="bo_sb", name="bo_sb")
                nc.sync.dma_start(bo_sb, bo_ext[:])
                bo_bc = wo_p.tile([P, D], F32, tag="bo_bc", name="bo_bc")
                nc.gpsimd.partition_broadcast(bo_bc[:], bo_sb[:])

                # ot_sb [p = h*64+d, src core, q]: single tile, filled by
                # three unpack DMAs; byte-range dep tracking means O-proj
                # column group mq only waits on the halves it actually reads.
                ot_sb = wo_p.tile([P, DC, QS], F16, name="ot_sb")

                def gate(dst_slice, src):
                    # 2-byte copy whose only purpose is scheduler placement:
                    # the list scheduler slots an instruction into an engine
                    # queue as soon as its deps are *scheduled* (collectives
                    # are free in its model), so anything downstream of an
                    # AllToAll would head-of-line-block the in-order queue.
                    # A write that depends on late attention output pins the
                    # unpack DMA (WAW) near its true ready time.
                    nc.vector.tensor_copy(dst_slice, src[0:1, 0:1])

                def unpack_ot0(onrm):
                    gate(ot_sb[0:1, 0, 0:1], onrm)
                    nc.scalar.dma_start(
                        ot_sb[0:HD, :, :],
                        a2a_out0[:].rearrange("s d q -> d s q"))

                def unpack_ot1(c, onrm):
                    gate(ot_sb[HD:HD + 1, 0,
                               c * (QS // 2):c * (QS // 2) + 1], onrm)
                    nc.scalar.dma_start(
                        ot_sb[HD:P, :, c * (QS // 2):(c + 1) * (QS // 2)],
                        a2a_out1[c][:].rearrange("s d q -> d s q"))

                # h1 attention in column halves: half 0 of both batches ->
                # first h1 AllToAll fires mid-attention; its O-proj rows
                # overlap the second half's collective.
                h1_post = {
                    (0, 1): {3: unpack_ot0},
                    (1, 1): {2: lambda o: unpack_ot1(0, o)},
                }
                last_onrm = [None]

                def save_onrm(o):
                    last_onrm[0] = o

                h1_post[(1, 1)][3] = save_onrm
                for c in range(2):
                    for b in range(2):
                        attn(1, b, a2a_in1[c], co=c * (QS // 2), cw=QS // 2,
                             post_qs=h1_post.get((c, b)))
                    nc.gpsimd.collective_compute(
                        "AllToAll", BYPASS,
                        ins=[a2a_in1[c][:].opt()],
                        outs=[a2a_out1[c][:].opt()],
                        replica_groups=RG)
                unpack_ot1(1, last_onrm[0])

                # ---------------- output projection ----------------
                # full-contraction matmuls, ordered so mq 0/1 (columns from
                # the first h1 collective) run while the second is in flight
                for mq in range(ROWS // P):
                    for oc in range(2):
                        psm = psV.tile([P, QS], F32, tag="mix", name="opsm")
                        for dc in range(DC):
                            nc.tensor.matmul(
                                psm,
                                lhsT=ot_sb[:, dc, mq * P:(mq + 1) * P],
                                rhs=wo_sb[:, dc, oc * QS:(oc + 1) * QS],
                                start=(dc == 0), stop=(dc == DC - 1))
                        o_t = nrm.tile([P, QS], F16, tag="ot", name="o_t")
                        with nc.allow_low_precision(
                                reason="f16 out, host converts"):
                            nc.vector.tensor_add(
                                out=o_t, in0=psm,
                                in1=bo_bc[:, oc * QS:(oc + 1) * QS])
                        nc.sync.dma_start(
                            out_ext[mq * P:(mq + 1) * P,
                                    oc * QS:(oc + 1) * QS], o_t)

    nc.finalize()
    return nc


def _chunked(w):
    # [D, n] -> [P, DC, n]: row r = c*P + p lands at [p, c]
    n = w.shape[1]
    return np.ascontiguousarray(w.reshape(DC, P, n).transpose(1, 0, 2))


def make_in_maps(inputs):
    f16 = np.float16
    x = np.asarray(inputs["x"], dtype=np.float32)
    # host-side shard layout: x transposed per batch, f16; weights in the
    # [partition, chunk, col] layout SBUF consumes (contiguous DMAs)
    xt = np.ascontiguousarray(x.transpose(0, 2, 1)).astype(f16)
    xt8 = np.ascontiguousarray(x.transpose(0, 2, 1)).astype(
        ml_dtypes.float8_e4m3fn)
    full_w = {k: np.asarray(inputs[k], np.float32).astype(f16)
              for k in ("wq", "wk", "wv", "wo")}
    full_b = {k: np.asarray(inputs[k], np.float32)
              for k in ("bq", "bk", "bv", "bo")}
    wo_r = _chunked(full_w["wo"])
    bo_r = full_b["bo"].reshape(1, D)
    in_maps = []
    for i in range(8):
        hs = i * HL
        m = {"xt": xt, "xt8": xt8,
             "wq": _chunked(full_w["wq"][:, hs:hs + HL]),
             "wk": _chunked(full_w["wk"][:, hs:hs + HL]),
             "wv": _chunked(full_w["wv"][:, hs:hs + HL]),
             "bq": np.ascontiguousarray(
                 full_b["bq"][hs:hs + HL].reshape(1, P).T),
             "bk": np.ascontiguousarray(
                 full_b["bk"][hs:hs + HL].reshape(1, P).T),
             "bv": full_b["bv"][hs:hs + HL].reshape(1, HL),
             "wo": wo_r,
             "bo": bo_r}
        in_maps.append(m)
    return in_maps


def kernel(**inputs):
    if "nc" not in _CACHE:
        _CACHE["nc"] = build_nc()
    nc = _CACHE["nc"]
    in_maps = make_in_maps(inputs)
    res = run_bass_kernel_spmd(nc, in_maps, core_ids=list(range(8)))
    out = np.empty((B, N, D), dtype=np.float32)
    for j in range(8):
        b, t = j // NQS, j % NQS
        out[b, t * ROWS:(t + 1) * ROWS] = res.results[j]["out"].astype(
            np.float32)
    return out
